# revision 20
# baseline (speedup 1.0000x reference)
"""MixLlamaMLP Trainium2 kernel.

y = (silu(x @ w_gate) * (x @ w_up)) @ w_down

Strategy: data-parallel over tokens across 8 NeuronCores (1024 tokens/core).
Per core, the intermediate h = silu(g)*u never touches DRAM: the I=11008
dimension is split into 3 passes (29/29/28 i-tiles); each pass computes its
h slice into SBUF, then immediately runs the down-projection from SBUF,
accumulating y across passes through a bf16 DRAM accumulator (DVE adds).
The down-projection computes y transposed ([h_out, tokens]) so w_down can
stream in 128-column chunks; the host un-transposes the gathered output.

Self-contained: hardcodes shapes B=4, S=2048, H=4096, I=11008.
"""
import os

import numpy as np

import concourse.bass as bass
import concourse.mybir as mybir
import concourse.tile as tile
from bass_rust import ScopedClock as _ScopedClock
from concourse.bass_utils import run_bass_kernel_spmd
from concourse.masks import make_identity

# ---------------------------------------------------------------------------
# Patch: this walrus build only accepts ONE sync-wait command per CTRL (Drain)
# instruction; Tile's kernel-tail drain carries one wait per logical proc.
# Split the waits across a chain of drain instructions (drain is idempotent).
_MAX_DRAIN_WAITS = 1


def _split_drain_and_barrier(self, tick_clock, wait_clock):
    nc = self.nc
    drain_inst = nc.sync.drain()
    wait_clock.add_sem_waits(
        drain_inst.ins, _ScopedClock({None: tick_clock.global_clock})
    )
    si = drain_inst.ins.sync_info
    waits = list(si.on_wait) if si is not None and si.on_wait else []
    if len(waits) > _MAX_DRAIN_WAITS:
        si.on_wait = waits[:_MAX_DRAIN_WAITS]
        rest = waits[_MAX_DRAIN_WAITS:]
        while rest:
            extra = nc.sync.drain()
            extra.ins.sync_info = mybir.SyncInfo(
                on_update=[], on_wait=rest[:_MAX_DRAIN_WAITS]
            )
            rest = rest[_MAX_DRAIN_WAITS:]
    nc.all_engine_barrier()
    assert self.sems is not None
    popped = nc._tile_sem_poison_stack.pop()
    assert popped is self._sem_poison
    nc.clear_and_free_semaphores(list(self.sems.allocated().values()))
    nc.all_engine_barrier()


tile.TileContext._drain_and_barrier = _split_drain_and_barrier


def _hoist_excess_waits(nc, max_waits=1):
    """Same walrus limitation, general case: any instruction may carry at most
    `max_waits` sync-wait commands. Hoist overflow waits onto same-engine NOPs
    inserted immediately before the instruction (engine streams are in-order,
    so the NOP blocking on the extra sems is equivalent)."""
    n_split = 0
    for fn in nc.m.functions:
        for blk in fn.blocks:
            insts = list(blk.instructions)
            new_insts = []
            changed = False
            for inst in insts:
                si = inst.sync_info
                waits = list(si.on_wait) if si is not None and si.on_wait else []
                if len(waits) > max_waits:
                    overflow = waits[: len(waits) - max_waits]
                    si.on_wait = waits[len(overflow) :]
                    for k in range(0, len(overflow), max_waits):
                        chunk = overflow[k : k + max_waits]
                        nop = mybir.InstNoOp(
                            name=nc.get_next_instruction_name(),
                            engine=inst.engine,
                            sync_info=mybir.SyncInfo(on_wait=chunk, on_update=[]),
                            bass_nofuse=True,
                        )
                        new_insts.append(nop)
                    changed = True
                    n_split += 1
                new_insts.append(inst)
            if changed:
                blk.instructions = new_insts
    return n_split
# ---------------------------------------------------------------------------

f32 = mybir.dt.float32
bf16 = mybir.dt.bfloat16

B, S, H, I = 4, 2048, 4096, 11008
NCORES = 8
M = (B * S) // NCORES  # 1024 tokens per core
P = 128
KT = H // P  # 32 contraction tiles for gate/up
IT = I // P  # 86 i tiles
MT = M // P  # 8 token tiles
TOKB = 512  # token free-dim block (psum free size)
NMB = M // TOKB  # 2
HT = H // P  # 32 h-out tiles for the down projection

# I-passes: h slice held in SBUF per pass
PASS_ITS = [(0, 29), (29, 58), (58, 86)]
ITP_MAX = 29

# Duty pacing: the chip power-throttles the PE to ~2.0 GHz under sustained
# ~100% matmul duty (P0). Insert small PE bubbles (chained DVE psum copies in
# each group's dependency path) from PACE_FROM_PASS onward to hold duty ~91%
# at 2.4 GHz instead.
PACE_FROM_PASS = 1
PACE_LEN = 1  # chained [128,128] f32 DVE copies, ~0.3us each (+latency)
PACE_EVERY = 2  # pace every Nth psum group


def _pace(nc, ps, prev_ps, n):
    """Delay the first matmul of the group writing `ps` by ~n*0.29us after
    the previous group's last matmul (via DVE copies chained off prev_ps)."""
    if prev_ps is None or n <= 0:
        return
    nc.vector.tensor_copy(ps[:, 0:P], prev_ps[:, 0:P])
    for j in range(1, n):
        nc.vector.tensor_copy(
            ps[:, (j % 4) * P : (j % 4) * P + P], ps[:, ((j - 1) % 4) * P : ((j - 1) % 4) * P + P]
        )


def _build_mlp(tc, x, wg, wu, wd, yT):
    nc = tc.nc

    with tc.tile_pool(name="dram", bufs=1, space="DRAM") as dram_pool:
        # bf16 accumulator for y^T across passes ([H, M] layout)
        y_acc = dram_pool.tile([H, M], bf16)

        with tc.tile_pool(name="xTp", bufs=1) as xT_pool, tc.tile_pool(
            name="wAp", bufs=3
        ) as wA_pool, tc.tile_pool(name="sgp", bufs=2) as sg_pool, tc.tile_pool(
            name="hp", bufs=1
        ) as h_pool, tc.tile_pool(name="wdp", bufs=2) as wd_pool, tc.tile_pool(
            name="ysp", bufs=2
        ) as ys_pool, tc.tile_pool(name="ypp", bufs=1) as yp_pool:
            xT = xT_pool.tile([P, KT, M], bf16)

            # -- Phase 0: x [M, H] f32 -> xT [H, M] bf16 --
            # x loads ride the idle HWDGE sync ring as f32 (keeps the SWDGE
            # queues free for weight prefetch); ACT casts, PE transposes.
            with tc.tile_pool(name="p0f", bufs=3) as x32_pool, tc.tile_pool(
                name="p0b", bufs=2
            ) as xbf_pool, tc.tile_pool(name="ident", bufs=1) as ident_pool, tc.tile_pool(
                name="p0psum", bufs=4, space="PSUM"
            ) as tr_psum:
                ident = ident_pool.tile([P, P], bf16)
                make_identity(nc, ident)
                HQ = H // 4
                KTQ = KT // 4
                for mt in range(MT):
                    for qq in range(4):
                        x_f = x32_pool.tile([P, HQ], f32)
                        nc.sync.dma_start(
                            x_f[:],
                            x[mt * P : (mt + 1) * P, qq * HQ : (qq + 1) * HQ],
                        )
                        x_b = xbf_pool.tile([P, HQ], bf16)
                        nc.scalar.copy(x_b[:], x_f[:])
                        for kq in range(KTQ):
                            kt = qq * KTQ + kq
                            tr = tr_psum.tile([P, P], bf16)
                            nc.tensor.transpose(
                                tr[:], x_b[:, kq * P : (kq + 1) * P], ident[:]
                            )
                            nc.vector.tensor_copy(
                                xT[:, kt, mt * P : (mt + 1) * P], tr[:]
                            )

            with tc.tile_pool(name="psA", bufs=2, space="PSUM") as psumA, tc.tile_pool(
                name="psB", bufs=4, space="PSUM"
            ) as psumB:
                wg_r = wg.rearrange("(kt p) i -> p kt i", p=P)
                wu_r = wu.rearrange("(kt p) i -> p kt i", p=P)
                wd_r = wd.rearrange("(it p) h -> p it h", p=P)
                y_acc_r = y_acc  # already [H, M]

                prev_ps = None
                grp = 0
                for pi, (it0, it1) in enumerate(PASS_ITS):
                    itp = it1 - it0
                    paced_pass = pi >= PACE_FROM_PASS
                    h_part = h_pool.tile([P, ITP_MAX, M], bf16)

                    # -- Stage A: h_part[:, il, :] = silu(g) * u  (bf16) --
                    for il in range(itp):
                        ig = it0 + il
                        isl = bass.ds(ig * P, P)
                        wbg = wA_pool.tile([P, KT, P], bf16)
                        nc.gpsimd.dma_start(wbg[:], wg_r[:, :, isl])
                        wbu = wA_pool.tile([P, KT, P], bf16)
                        nc.gpsimd.dma_start(wbu[:], wu_r[:, :, isl])
                        for mb in range(NMB):
                            msl = bass.ds(mb * TOKB, TOKB)
                            pg = psumA.tile([P, TOKB], f32, tag="pg")
                            pu = psumA.tile([P, TOKB], f32, tag="pu")
                            grp += 1
                            if paced_pass and grp % PACE_EVERY == 0:
                                _pace(nc, pg, prev_ps, PACE_LEN)
                            for kt in range(KT):
                                nc.tensor.matmul(
                                    pg[:],
                                    wbg[:, kt, :],
                                    xT[:, kt, msl],
                                    start=(kt == 0),
                                    stop=(kt == KT - 1),
                                )
                            grp += 1
                            if paced_pass and grp % PACE_EVERY == 0:
                                _pace(nc, pu, pg, PACE_LEN)
                            for kt in range(KT):
                                nc.tensor.matmul(
                                    pu[:],
                                    wbu[:, kt, :],
                                    xT[:, kt, msl],
                                    start=(kt == 0),
                                    stop=(kt == KT - 1),
                                )
                            prev_ps = pu
                            sg = sg_pool.tile([P, TOKB], bf16)
                            nc.scalar.activation(
                                sg[:], pg[:], mybir.ActivationFunctionType.Silu
                            )
                            nc.vector.tensor_mul(
                                out=h_part[:, il, msl], in0=sg[:], in1=pu[:]
                            )

                    # -- Stage B: y^T[ho, :] += sum_il wd[il, ho]^T h[il, :] --
                    for hot in range(HT):
                        hsl = bass.ds(hot * P, P)
                        wdc = wd_pool.tile([P, ITP_MAX, P], bf16)
                        nc.gpsimd.dma_start(
                            wdc[:, :itp, :], wd_r[:, it0:it1, hsl]
                        )
                        if pi > 0:
                            ypv = yp_pool.tile([P, M], bf16)
                            nc.sync.dma_start(ypv[:], y_acc_r[hsl, :])
                        for mb in range(NMB):
                            msl = bass.ds(mb * TOKB, TOKB)
                            yp = psumB.tile([P, TOKB], f32)
                            grp += 1
                            if paced_pass and grp % PACE_EVERY == 0:
                                _pace(nc, yp, prev_ps, PACE_LEN)
                            for il in range(itp):
                                nc.tensor.matmul(
                                    yp[:],
                                    wdc[:, il, :],
                                    h_part[:, il, msl],
                                    start=(il == 0),
                                    stop=(il == itp - 1),
                                )
                            prev_ps = yp
                            ys = ys_pool.tile([P, TOKB], bf16)
                            if pi == 0:
                                nc.scalar.copy(ys[:], yp[:])
                            else:
                                nc.vector.tensor_add(
                                    out=ys[:], in0=yp[:], in1=ypv[:, msl]
                                )
                            if pi < 2:
                                nc.scalar.dma_start(y_acc_r[hsl, msl], ys[:])
                            else:
                                # cast bf16 -> f32 in flight (SWDGE)
                                nc.gpsimd.dma_start(yT[hsl, msl], ys[:])


_NC_CACHE = None


def _build():
    global _NC_CACHE
    if _NC_CACHE is not None:
        return _NC_CACHE
    nc = bass.Bass(num_swdge_queues=4)
    x = nc.dram_tensor("x", [M, H], f32, kind="ExternalInput")
    wg = nc.dram_tensor("w_gate", [H, I], f32, kind="ExternalInput")
    wu = nc.dram_tensor("w_up", [H, I], f32, kind="ExternalInput")
    wd = nc.dram_tensor("w_down", [I, H], f32, kind="ExternalInput")
    yT = nc.dram_tensor("yT", [H, M], f32, kind="ExternalOutput")
    with tile.TileContext(nc) as tc:
        _build_mlp(tc, x, wg, wu, wd, yT)
    _hoist_excess_waits(nc)
    _NC_CACHE = nc
    return nc


LAST_RESULTS = None


def kernel(x, w_gate, w_up, w_down):
    global LAST_RESULTS
    x = np.ascontiguousarray(np.asarray(x, dtype=np.float32)).reshape(B * S, H)
    w_gate = np.ascontiguousarray(np.asarray(w_gate, dtype=np.float32))
    w_up = np.ascontiguousarray(np.asarray(w_up, dtype=np.float32))
    w_down = np.ascontiguousarray(np.asarray(w_down, dtype=np.float32))

    nc = _build()
    in_maps = [
        {
            "x": x[c * M : (c + 1) * M],
            "w_gate": w_gate,
            "w_up": w_up,
            "w_down": w_down,
        }
        for c in range(NCORES)
    ]
    trace = os.environ.get("KERNEL_TRACE") == "1"
    res = run_bass_kernel_spmd(
        nc, in_maps, core_ids=list(range(NCORES)), trace=trace
    )
    LAST_RESULTS = res
    if res.exec_time_ns is not None:
        print(f"HW exec time: {res.exec_time_ns} ns")
    y = np.concatenate(
        [np.ascontiguousarray(r["yT"].T) for r in res.results], axis=0
    )
    return y.reshape(B, S, H)


# revision 21
# speedup vs baseline: 1.1215x; 1.1215x over previous
"""MixLlamaMLP Trainium2 kernel.

y = (silu(x @ w_gate) * (x @ w_up)) @ w_down

Strategy: data-parallel over tokens across 8 NeuronCores (1024 tokens/core);
each core runs the full MLP on its token slice in bf16 (fp32 accumulate),
streaming all three weight matrices from HBM exactly once. No collectives.

Self-contained: hardcodes shapes B=4, S=2048, H=4096, I=11008.
"""
import os

import numpy as np

import concourse.bass as bass
import concourse.mybir as mybir
import concourse.tile as tile
from bass_rust import ScopedClock as _ScopedClock
from concourse.bass_utils import run_bass_kernel_spmd
from concourse.masks import make_identity

# ---------------------------------------------------------------------------
# Patch: this walrus build only accepts ONE sync-wait command per CTRL (Drain)
# instruction; Tile's kernel-tail drain carries one wait per logical proc.
# Split the waits across a chain of drain instructions (drain is idempotent).
_MAX_DRAIN_WAITS = 1


def _split_drain_and_barrier(self, tick_clock, wait_clock):
    nc = self.nc
    drain_inst = nc.sync.drain()
    wait_clock.add_sem_waits(
        drain_inst.ins, _ScopedClock({None: tick_clock.global_clock})
    )
    si = drain_inst.ins.sync_info
    waits = list(si.on_wait) if si is not None and si.on_wait else []
    if len(waits) > _MAX_DRAIN_WAITS:
        si.on_wait = waits[:_MAX_DRAIN_WAITS]
        rest = waits[_MAX_DRAIN_WAITS:]
        while rest:
            extra = nc.sync.drain()
            extra.ins.sync_info = mybir.SyncInfo(
                on_update=[], on_wait=rest[:_MAX_DRAIN_WAITS]
            )
            rest = rest[_MAX_DRAIN_WAITS:]
    nc.all_engine_barrier()
    assert self.sems is not None
    popped = nc._tile_sem_poison_stack.pop()
    assert popped is self._sem_poison
    nc.clear_and_free_semaphores(list(self.sems.allocated().values()))
    nc.all_engine_barrier()


tile.TileContext._drain_and_barrier = _split_drain_and_barrier


def _hoist_excess_waits(nc, max_waits=1):
    """Same walrus limitation, general case: any instruction may carry at most
    `max_waits` sync-wait commands. Hoist overflow waits onto same-engine NOPs
    inserted immediately before the instruction (engine streams are in-order,
    so the NOP blocking on the extra sems is equivalent)."""
    n_split = 0
    for fn in nc.m.functions:
        for blk in fn.blocks:
            insts = list(blk.instructions)
            new_insts = []
            changed = False
            for inst in insts:
                si = inst.sync_info
                waits = list(si.on_wait) if si is not None and si.on_wait else []
                if len(waits) > max_waits:
                    overflow = waits[: len(waits) - max_waits]
                    si.on_wait = waits[len(overflow) :]
                    for k in range(0, len(overflow), max_waits):
                        chunk = overflow[k : k + max_waits]
                        nop = mybir.InstNoOp(
                            name=nc.get_next_instruction_name(),
                            engine=inst.engine,
                            sync_info=mybir.SyncInfo(on_wait=chunk, on_update=[]),
                            bass_nofuse=True,
                        )
                        new_insts.append(nop)
                    changed = True
                    n_split += 1
                new_insts.append(inst)
            if changed:
                blk.instructions = new_insts
    return n_split
# ---------------------------------------------------------------------------

f32 = mybir.dt.float32
bf16 = mybir.dt.bfloat16

B, S, H, I = 4, 2048, 4096, 11008
NCORES = 8
M = (B * S) // NCORES  # 1024 tokens per core
P = 128
KT = H // P  # 32 contraction tiles for gate/up
IT = I // P  # 86 i tiles
MT = M // P  # 8 token tiles
TOKB = 512  # token free-dim block (stage A psum free size)
NMB = M // TOKB  # 2
ICH = 256  # i-chunk width for stage-A weight streaming
NICH = I // ICH  # 43
HOB = 512  # h-out block (stage B psum free size)
NHO = H // HOB  # 8
BCH = 11  # i-tiles per stage-B chunk (86 = 7*11 + 9)


def _stage_b_chunks():
    chunks = []
    c0 = 0
    while c0 < IT:
        clen = min(BCH, IT - c0)
        chunks.append((c0, clen))
        c0 += clen
    return chunks


def _build_mlp(tc, x, wg, wu, wd, y):
    nc = tc.nc

    with tc.tile_pool(name="dram", bufs=1, space="DRAM") as dram_pool:
        h_dram = dram_pool.tile([IT, P, M], bf16)

        # Stage-B SBUF pools open first (outermost) so their loads can overlap
        # the tail of stage A (no address reuse against stage-A pools).
        with tc.tile_pool(name="hBp", bufs=2) as hB_pool, tc.tile_pool(
            name="wdbp", bufs=2
        ) as wd_bfp, tc.tile_pool(name="ysb", bufs=2) as y_pool:
            with tc.tile_pool(name="xTp", bufs=1) as xT_pool:
                xT = xT_pool.tile([P, KT, M], bf16)

                # -- Phase 0: x [M, H] f32 -> xT [H, M] bf16 --
                # gpsimd DMA casts f32->bf16 in flight; PE transposes bf16.
                with tc.tile_pool(name="p0", bufs=2) as x_pool, tc.tile_pool(
                    name="ident", bufs=1
                ) as ident_pool, tc.tile_pool(
                    name="p0psum", bufs=4, space="PSUM"
                ) as tr_psum:
                    ident = ident_pool.tile([P, P], bf16)
                    make_identity(nc, ident)
                    for mt in range(MT):
                        x_sb = x_pool.tile([P, H], bf16)
                        nc.gpsimd.dma_start(x_sb[:], x[mt * P : (mt + 1) * P, :])
                        for kt in range(KT):
                            tr = tr_psum.tile([P, P], bf16)
                            nc.tensor.transpose(
                                tr[:], x_sb[:, kt * P : (kt + 1) * P], ident[:]
                            )
                            nc.vector.tensor_copy(
                                xT[:, kt, mt * P : (mt + 1) * P], tr[:]
                            )

                # -- Stage A: up/gate matmuls + silu*up -> h_dram (bf16) --
                # weights arrive bf16 via gpsimd cast-DMA; h writes go out on
                # the ACT HWDGE queue so SP only carries stage-B h loads.
                with tc.tile_pool(name="wAb", bufs=2) as wA_bf, tc.tile_pool(
                    name="sgp", bufs=2
                ) as sg_pool, tc.tile_pool(name="hAp", bufs=3) as hA_pool, tc.tile_pool(
                    name="psA", bufs=2, space="PSUM"
                ) as psumA:
                    wg_r = wg.rearrange("(kt p) i -> p kt i", p=P)
                    wu_r = wu.rearrange("(kt p) i -> p kt i", p=P)
                    for ic in range(NICH):
                        isl = bass.ds(ic * ICH, ICH)
                        wbg = wA_bf.tile([P, KT, ICH], bf16, tag="wg")
                        nc.gpsimd.dma_start(wbg[:], wg_r[:, :, isl])
                        wbu = wA_bf.tile([P, KT, ICH], bf16, tag="wu")
                        nc.gpsimd.dma_start(wbu[:], wu_r[:, :, isl])
                        for it in range(ICH // P):
                            i_glob = ic * (ICH // P) + it
                            for mb in range(NMB):
                                pg = psumA.tile([P, TOKB], f32, tag="pg")
                                pu = psumA.tile([P, TOKB], f32, tag="pu")
                                msl = bass.ds(mb * TOKB, TOKB)
                                for kt in range(KT):
                                    nc.tensor.matmul(
                                        pg[:],
                                        wbg[:, kt, it * P : (it + 1) * P],
                                        xT[:, kt, msl],
                                        start=(kt == 0),
                                        stop=(kt == KT - 1),
                                    )
                                for kt in range(KT):
                                    nc.tensor.matmul(
                                        pu[:],
                                        wbu[:, kt, it * P : (it + 1) * P],
                                        xT[:, kt, msl],
                                        start=(kt == 0),
                                        stop=(kt == KT - 1),
                                    )
                                sg = sg_pool.tile([P, TOKB], f32)
                                nc.scalar.activation(
                                    sg[:], pg[:], mybir.ActivationFunctionType.Silu
                                )
                                ht = hA_pool.tile([P, TOKB], bf16)
                                nc.vector.tensor_mul(out=ht[:], in0=sg[:], in1=pu[:])
                                nc.scalar.dma_start(h_dram[i_glob, :, msl], ht[:])

            # -- Stage B: y = h @ w_down, streaming h + w_down from DRAM --
            with tc.tile_pool(name="psB", bufs=1, space="PSUM") as psumB:
                wd_r = wd.rearrange("(it p) h -> p it h", p=P)
                chunks = _stage_b_chunks()
                for ho in range(NHO):
                    hosl = bass.ds(ho * HOB, HOB)
                    ypsums = []
                    for tp in range(MT):
                        yp = psumB.tile([P, HOB], f32, tag=f"y{tp}")
                        ypsums.append(yp)
                    for c0, clen in chunks:
                        hc = hB_pool.tile([P, BCH, M], bf16, tag="hc")
                        nc.sync.dma_start(
                            hc[:, :clen, :],
                            h_dram[c0 : c0 + clen].rearrange("it p m -> p it m"),
                        )
                        wdb = wd_bfp.tile([P, BCH, HOB], bf16, tag="wdb")
                        nc.gpsimd.dma_start(
                            wdb[:, :clen, :], wd_r[:, c0 : c0 + clen, hosl]
                        )
                        # tp-outer: keep consecutive matmuls on the same PSUM
                        # bank (per-MM bank cycling causes PE micro-stalls)
                        for tp in range(MT):
                            for il in range(clen):
                                i_glob = c0 + il
                                nc.tensor.matmul(
                                    ypsums[tp][:],
                                    hc[:, il, tp * P : (tp + 1) * P],
                                    wdb[:, il, :],
                                    start=(i_glob == 0),
                                    stop=(i_glob == IT - 1),
                                )
                    for tp in range(MT):
                        yt = y_pool.tile([P, HOB], f32)
                        nc.scalar.copy(yt[:], ypsums[tp][:])
                        nc.sync.dma_start(y[tp * P : (tp + 1) * P, hosl], yt[:])


_NC_CACHE = None


def _build():
    global _NC_CACHE
    if _NC_CACHE is not None:
        return _NC_CACHE
    nc = bass.Bass(num_swdge_queues=4)
    x = nc.dram_tensor("x", [M, H], f32, kind="ExternalInput")
    wg = nc.dram_tensor("w_gate", [H, I], f32, kind="ExternalInput")
    wu = nc.dram_tensor("w_up", [H, I], f32, kind="ExternalInput")
    wd = nc.dram_tensor("w_down", [I, H], f32, kind="ExternalInput")
    y = nc.dram_tensor("y", [M, H], f32, kind="ExternalOutput")
    with tile.TileContext(nc) as tc:
        _build_mlp(tc, x, wg, wu, wd, y)
    _hoist_excess_waits(nc)
    _NC_CACHE = nc
    return nc


LAST_RESULTS = None


def kernel(x, w_gate, w_up, w_down):
    global LAST_RESULTS
    x = np.ascontiguousarray(np.asarray(x, dtype=np.float32)).reshape(B * S, H)
    w_gate = np.ascontiguousarray(np.asarray(w_gate, dtype=np.float32))
    w_up = np.ascontiguousarray(np.asarray(w_up, dtype=np.float32))
    w_down = np.ascontiguousarray(np.asarray(w_down, dtype=np.float32))

    nc = _build()
    in_maps = [
        {
            "x": x[c * M : (c + 1) * M],
            "w_gate": w_gate,
            "w_up": w_up,
            "w_down": w_down,
        }
        for c in range(NCORES)
    ]
    trace = os.environ.get("KERNEL_TRACE") == "1"
    res = run_bass_kernel_spmd(
        nc, in_maps, core_ids=list(range(NCORES)), trace=trace
    )
    LAST_RESULTS = res
    if res.exec_time_ns is not None:
        print(f"HW exec time: {res.exec_time_ns} ns")
    y = np.concatenate([r["y"] for r in res.results], axis=0)
    return y.reshape(B, S, H)



# revision 24
# speedup vs baseline: 1.1433x; 1.0195x over previous
"""MixLlamaMLP Trainium2 kernel.

y = (silu(x @ w_gate) * (x @ w_up)) @ w_down

Strategy: data-parallel over tokens across 8 NeuronCores (1024 tokens/core);
each core runs the full MLP on its token slice in bf16 (fp32 accumulate),
streaming all three weight matrices from HBM exactly once. No collectives.

Self-contained: hardcodes shapes B=4, S=2048, H=4096, I=11008.
"""
import os

import numpy as np

import concourse.bass as bass
import concourse.mybir as mybir
import concourse.tile as tile
from bass_rust import ScopedClock as _ScopedClock
from concourse.bass_utils import run_bass_kernel_spmd
from concourse.masks import make_identity

# ---------------------------------------------------------------------------
# Patch: this walrus build only accepts ONE sync-wait command per CTRL (Drain)
# instruction; Tile's kernel-tail drain carries one wait per logical proc.
# Split the waits across a chain of drain instructions (drain is idempotent).
_MAX_DRAIN_WAITS = 1


def _split_drain_and_barrier(self, tick_clock, wait_clock):
    nc = self.nc
    drain_inst = nc.sync.drain()
    wait_clock.add_sem_waits(
        drain_inst.ins, _ScopedClock({None: tick_clock.global_clock})
    )
    si = drain_inst.ins.sync_info
    waits = list(si.on_wait) if si is not None and si.on_wait else []
    if len(waits) > _MAX_DRAIN_WAITS:
        si.on_wait = waits[:_MAX_DRAIN_WAITS]
        rest = waits[_MAX_DRAIN_WAITS:]
        while rest:
            extra = nc.sync.drain()
            extra.ins.sync_info = mybir.SyncInfo(
                on_update=[], on_wait=rest[:_MAX_DRAIN_WAITS]
            )
            rest = rest[_MAX_DRAIN_WAITS:]
    nc.all_engine_barrier()
    assert self.sems is not None
    popped = nc._tile_sem_poison_stack.pop()
    assert popped is self._sem_poison
    nc.clear_and_free_semaphores(list(self.sems.allocated().values()))
    nc.all_engine_barrier()


tile.TileContext._drain_and_barrier = _split_drain_and_barrier


def _hoist_excess_waits(nc, max_waits=1):
    """Same walrus limitation, general case: any instruction may carry at most
    `max_waits` sync-wait commands. Hoist overflow waits onto same-engine NOPs
    inserted immediately before the instruction (engine streams are in-order,
    so the NOP blocking on the extra sems is equivalent)."""
    n_split = 0
    for fn in nc.m.functions:
        for blk in fn.blocks:
            insts = list(blk.instructions)
            new_insts = []
            changed = False
            for inst in insts:
                si = inst.sync_info
                waits = list(si.on_wait) if si is not None and si.on_wait else []
                if len(waits) > max_waits:
                    overflow = waits[: len(waits) - max_waits]
                    si.on_wait = waits[len(overflow) :]
                    for k in range(0, len(overflow), max_waits):
                        chunk = overflow[k : k + max_waits]
                        nop = mybir.InstNoOp(
                            name=nc.get_next_instruction_name(),
                            engine=inst.engine,
                            sync_info=mybir.SyncInfo(on_wait=chunk, on_update=[]),
                            bass_nofuse=True,
                        )
                        new_insts.append(nop)
                    changed = True
                    n_split += 1
                new_insts.append(inst)
            if changed:
                blk.instructions = new_insts
    return n_split
# ---------------------------------------------------------------------------

f32 = mybir.dt.float32
bf16 = mybir.dt.bfloat16

B, S, H, I = 4, 2048, 4096, 11008
NCORES = 8
M = (B * S) // NCORES  # 1024 tokens per core
P = 128
KT = H // P  # 32 contraction tiles for gate/up
IT = I // P  # 86 i tiles
MT = M // P  # 8 token tiles
TOKB = 512  # token free-dim block (stage A psum free size)
NMB = M // TOKB  # 2
ICH = 128  # i-chunk width for stage-A weight streaming
NICH = I // ICH  # 86
HOB = 512  # h-out block (stage B psum free size)
NHO = H // HOB  # 8
BCH = 11  # i-tiles per stage-B chunk (86 = 7*11 + 9)


def _stage_b_chunks():
    chunks = []
    c0 = 0
    while c0 < IT:
        clen = min(BCH, IT - c0)
        chunks.append((c0, clen))
        c0 += clen
    return chunks


def _build_mlp(tc, x, wg, wu, wd, y):
    nc = tc.nc

    with tc.tile_pool(name="dram", bufs=1, space="DRAM") as dram_pool:
        h_dram = dram_pool.tile([IT, P, M], bf16)

        # Stage-B SBUF pools open first (outermost) so their loads can overlap
        # the tail of stage A (no address reuse against stage-A pools).
        with tc.tile_pool(name="hBp", bufs=3) as hB_pool, tc.tile_pool(
            name="wdbp", bufs=3
        ) as wd_bfp, tc.tile_pool(name="ysb", bufs=2) as y_pool:
            with tc.tile_pool(name="xTp", bufs=1) as xT_pool:
                xT = xT_pool.tile([P, KT, M], bf16)

                # -- Phase 0: x [M, H] f32 -> xT [H, M] bf16 --
                # gpsimd DMA casts f32->bf16 in flight; PE transposes bf16.
                with tc.tile_pool(name="p0", bufs=2) as x_pool, tc.tile_pool(
                    name="ident", bufs=1
                ) as ident_pool, tc.tile_pool(
                    name="p0psum", bufs=4, space="PSUM"
                ) as tr_psum:
                    ident = ident_pool.tile([P, P], bf16)
                    make_identity(nc, ident)
                    for mt in range(MT):
                        x_sb = x_pool.tile([P, H], bf16)
                        nc.gpsimd.dma_start(x_sb[:], x[mt * P : (mt + 1) * P, :])
                        for kt in range(KT):
                            tr = tr_psum.tile([P, P], bf16)
                            nc.tensor.transpose(
                                tr[:], x_sb[:, kt * P : (kt + 1) * P], ident[:]
                            )
                            nc.vector.tensor_copy(
                                xT[:, kt, mt * P : (mt + 1) * P], tr[:]
                            )

                # -- Stage A: up/gate matmuls + silu*up -> h_dram (bf16) --
                # weights arrive bf16 via gpsimd cast-DMA; h writes go out on
                # the ACT HWDGE queue so SP only carries stage-B h loads.
                with tc.tile_pool(name="wAb", bufs=2) as wA_bf, tc.tile_pool(
                    name="sgp", bufs=2
                ) as sg_pool, tc.tile_pool(name="hAp", bufs=3) as hA_pool, tc.tile_pool(
                    name="psA", bufs=2, space="PSUM"
                ) as psumA:
                    wg_r = wg.rearrange("(kt p) i -> p kt i", p=P)
                    wu_r = wu.rearrange("(kt p) i -> p kt i", p=P)
                    for ic in range(NICH):
                        isl = bass.ds(ic * ICH, ICH)
                        wbg = wA_bf.tile([P, KT, ICH], bf16, tag="wg")
                        nc.gpsimd.dma_start(wbg[:], wg_r[:, :, isl])
                        wbu = wA_bf.tile([P, KT, ICH], bf16, tag="wu")
                        nc.gpsimd.dma_start(wbu[:], wu_r[:, :, isl])
                        for it in range(ICH // P):
                            i_glob = ic * (ICH // P) + it
                            for mb in range(NMB):
                                pg = psumA.tile([P, TOKB], f32, tag="pg")
                                pu = psumA.tile([P, TOKB], f32, tag="pu")
                                msl = bass.ds(mb * TOKB, TOKB)
                                for kt in range(KT):
                                    nc.tensor.matmul(
                                        pg[:],
                                        wbg[:, kt, it * P : (it + 1) * P],
                                        xT[:, kt, msl],
                                        start=(kt == 0),
                                        stop=(kt == KT - 1),
                                    )
                                for kt in range(KT):
                                    nc.tensor.matmul(
                                        pu[:],
                                        wbu[:, kt, it * P : (it + 1) * P],
                                        xT[:, kt, msl],
                                        start=(kt == 0),
                                        stop=(kt == KT - 1),
                                    )
                                sg = sg_pool.tile([P, TOKB], f32)
                                nc.scalar.activation(
                                    sg[:], pg[:], mybir.ActivationFunctionType.Silu
                                )
                                ht = hA_pool.tile([P, TOKB], bf16)
                                nc.vector.tensor_mul(out=ht[:], in0=sg[:], in1=pu[:])
                                nc.scalar.dma_start(h_dram[i_glob, :, msl], ht[:])

            # -- Stage B: y = h @ w_down, streaming h + w_down from DRAM --
            with tc.tile_pool(name="psB", bufs=1, space="PSUM") as psumB:
                wd_r = wd.rearrange("(it p) h -> p it h", p=P)
                chunks = _stage_b_chunks()
                for ho in range(NHO):
                    hosl = bass.ds(ho * HOB, HOB)
                    ypsums = []
                    for tp in range(MT):
                        yp = psumB.tile([P, HOB], f32, tag=f"y{tp}")
                        ypsums.append(yp)
                    for c0, clen in chunks:
                        hc = hB_pool.tile([P, BCH, M], bf16, tag="hc")
                        nc.sync.dma_start(
                            hc[:, :clen, :],
                            h_dram[c0 : c0 + clen].rearrange("it p m -> p it m"),
                        )
                        wdb = wd_bfp.tile([P, BCH, HOB], bf16, tag="wdb")
                        nc.gpsimd.dma_start(
                            wdb[:, :clen, :], wd_r[:, c0 : c0 + clen, hosl]
                        )
                        # tp-outer: keep consecutive matmuls on the same PSUM
                        # bank (per-MM bank cycling causes PE micro-stalls)
                        for tp in range(MT):
                            for il in range(clen):
                                i_glob = c0 + il
                                nc.tensor.matmul(
                                    ypsums[tp][:],
                                    hc[:, il, tp * P : (tp + 1) * P],
                                    wdb[:, il, :],
                                    start=(i_glob == 0),
                                    stop=(i_glob == IT - 1),
                                )
                    for tp in range(MT):
                        yt = y_pool.tile([P, HOB], f32)
                        nc.scalar.copy(yt[:], ypsums[tp][:])
                        # ACT ring is idle in stage B; keep SP for h loads only
                        nc.scalar.dma_start(y[tp * P : (tp + 1) * P, hosl], yt[:])


_NC_CACHE = None


def _build():
    global _NC_CACHE
    if _NC_CACHE is not None:
        return _NC_CACHE
    nc = bass.Bass(num_swdge_queues=4)
    x = nc.dram_tensor("x", [M, H], f32, kind="ExternalInput")
    wg = nc.dram_tensor("w_gate", [H, I], f32, kind="ExternalInput")
    wu = nc.dram_tensor("w_up", [H, I], f32, kind="ExternalInput")
    wd = nc.dram_tensor("w_down", [I, H], f32, kind="ExternalInput")
    y = nc.dram_tensor("y", [M, H], f32, kind="ExternalOutput")
    with tile.TileContext(nc) as tc:
        _build_mlp(tc, x, wg, wu, wd, y)
    _hoist_excess_waits(nc)
    _NC_CACHE = nc
    return nc


LAST_RESULTS = None


def kernel(x, w_gate, w_up, w_down):
    global LAST_RESULTS
    x = np.ascontiguousarray(np.asarray(x, dtype=np.float32)).reshape(B * S, H)
    w_gate = np.ascontiguousarray(np.asarray(w_gate, dtype=np.float32))
    w_up = np.ascontiguousarray(np.asarray(w_up, dtype=np.float32))
    w_down = np.ascontiguousarray(np.asarray(w_down, dtype=np.float32))

    nc = _build()
    in_maps = [
        {
            "x": x[c * M : (c + 1) * M],
            "w_gate": w_gate,
            "w_up": w_up,
            "w_down": w_down,
        }
        for c in range(NCORES)
    ]
    trace = os.environ.get("KERNEL_TRACE") == "1"
    res = run_bass_kernel_spmd(
        nc, in_maps, core_ids=list(range(NCORES)), trace=trace
    )
    LAST_RESULTS = res
    if res.exec_time_ns is not None:
        print(f"HW exec time: {res.exec_time_ns} ns")
    y = np.concatenate([r["y"] for r in res.results], axis=0)
    return y.reshape(B, S, H)

